# revision 3
# baseline (speedup 1.0000x reference)
"""Trainium2 Bass kernel: single attention head (B=8, S=2048, E=1024, H=64).

Sharding: data-parallel over batch -- each of the 8 NeuronCores computes one
batch element's full attention. No collectives; every HBM byte read once.

v10 design (exp-metronome schedule, k-early/v-late stream):
  - Trace evidence from v9 (96.3us): ScalarE exp (32 x 1113ns = 35.6us) is
    the steady-state rate limiter, but it idled 28.7->84.9us with 20us of
    gaps because the DMA stream order (k,v interleaved) and program order
    back-loaded half the score work after the stream finished. Also the
    vaug DMA-transposes sat FIFO-behind the whole input stream on the sync
    HWDGE ring (executed ~64us!), and their dispatches cost 1.2us each on
    an engine queue.
  - v10 stream order (sync ring, FIFO): xq0a, k0, xq0b, k1, k2, xq1, v0,
    k3, v1, v2, v3. k-blocks early: scores+exp start ~19us and ScalarE
    runs back-to-back. v-blocks late: AV matmuls are deferrable PE filler.
  - exp slab order follows data arrival: [t0-3 q0][t4-7 q0][t8-11 q0]
    [t0-7 q1 row-paired][t12-15 q0 x t8-11 q1 row-paired][t12-15 q1].
  - v^T -> vaug transpose now via PE transpose + DVE copy (on-chip, no
    DMA ring involvement). Output DMAs moved to the sync ring so the
    scalar engine's stream is pure ACTIVATE in the critical window.
  - k/v projections pair col-tiled ACROSS blocks (k1 row-h64 || k2 row-h0
    would be ideal but k2 arrives later; pairs used: k3||v0, v1||v2) to
    halve PE cost where data timing allows.
  - Everything else as v9: duplicated-Wq qproj ([128,128] stationary, one
    matmul per chunk+seg), scores transposed (keys on partitions), rowsum
    rides the ones column of the AV stationary, bk cancels in softmax,
    bq/bv fold into projection evacuations, finalize via PE transpose +
    DVE reciprocal-scale, batched f32 output DMA.

PSUM: 2 x 2-bank rotating slots + 4-bank AV accumulator = 8 banks.
"""

import numpy as np

import concourse.bass as bass  # noqa: F401  (engine namespaces live on nc)
import concourse.mybir as mybir
import concourse.tile as tile
from concourse import bacc
from concourse.bass_utils import run_bass_kernel_spmd
from concourse.masks import make_identity

B, S, E, H = 8, 2048, 1024, 64
EC = E // 128    # contraction chunks (128 partitions each)
KB = 512         # kv block columns
NKB = S // KB    # 4 kv blocks
NT = S // 128    # key tiles
F16 = mybir.dt.float16
F32 = mybir.dt.float32

_CACHE = {}


def _build_nc():
    nc = bacc.Bacc(None)
    xq = nc.declare_dram_parameter("xq", [128, 2, EC, S // 2], F16, isOutput=False)
    xk = nc.declare_dram_parameter("xk", [128, NKB, EC, KB], F16, isOutput=False)
    xv = nc.declare_dram_parameter("xv", [128, NKB, EC, KB], F16, isOutput=False)
    wqd = nc.declare_dram_parameter("wqd", [128, EC, 128], F16, isOutput=False)
    wk = nc.declare_dram_parameter("wk", [128, EC, H], F16, isOutput=False)
    wv = nc.declare_dram_parameter("wv", [128, EC, H], F16, isOutput=False)
    bq = nc.declare_dram_parameter("bq", [128, 1], F32, isOutput=False)
    bv = nc.declare_dram_parameter("bv", [128, 1], F32, isOutput=False)
    out = nc.declare_dram_parameter("out", [S, H], F32, isOutput=True)

    Exp = mybir.ActivationFunctionType.Exp

    with tile.TileContext(nc) as tc:
        with tc.tile_pool(name="const", bufs=1) as const, \
             tc.tile_pool(name="xkp", bufs=4) as xkp, \
             tc.tile_pool(name="xvp", bufs=4) as xvp, \
             tc.tile_pool(name="ptp", bufs=16) as ptp, \
             tc.tile_pool(name="vtp", bufs=2) as vtp, \
             tc.tile_pool(name="p5sb", bufs=2) as p5sb, \
             tc.tile_pool(name="psp", bufs=2, space="PSUM") as psp, \
             tc.tile_pool(name="oap", bufs=1, space="PSUM") as oap:

            # ---- constants on the scalar HWDGE ring (early, tiny) ----
            wqd_t = const.tile([128, EC, 128], F16, name="wqd_t")
            nc.scalar.dma_start(out=wqd_t[:], in_=wqd[:])
            wk_t = const.tile([128, EC, H], F16, name="wk_t")
            nc.scalar.dma_start(out=wk_t[:], in_=wk[:])
            wv_t = const.tile([128, EC, H], F16, name="wv_t")
            nc.scalar.dma_start(out=wv_t[:], in_=wv[:])
            bq_t = const.tile([128, 1], F32, name="bq_t")
            nc.scalar.dma_start(out=bq_t[:], in_=bq[:])
            bv_t = const.tile([128, 1], F32, name="bv_t")
            nc.scalar.dma_start(out=bv_t[:], in_=bv[:])

            qt = const.tile([128, S], F16, name="qt")     # q^T in BOTH halves
            kt = const.tile([128, S], F16, name="kt")     # k^T: half (jb%2)
            xqt = const.tile([128, EC, S], F16, name="xqt")
            vaug = const.tile([128, NT, 80], F16, name="vaug")
            oasb = const.tile([65, S], F16, name="oasb")
            ident = const.tile([128, 128], F16, name="ident")
            osb_all = const.tile([128, NT, H], F32, name="osb_all")

            make_identity(nc, ident[:])
            nc.vector.memset(vaug[:, :, 64], 1.0)

            oa = oap.tile([65, S], F32, name="oa")        # AV accumulator

            def slot(name):
                return psp.tile([128, 1024], F32, tag="ps", name=name)

            # ---- input DMAs (sync HWDGE FIFO -- executes in this order) ----
            xkts, xvts = [], []

            def fetch(which, jb):
                if which == "k":
                    xt = xkp.tile([128, EC, KB], F16, tag="xk", name=f"xkt{jb}")
                    nc.sync.dma_start(out=xt[:], in_=xk[:, jb])
                    xkts.append(xt)
                else:
                    xt = xvp.tile([128, EC, KB], F16, tag="xv", name=f"xvt{jb}")
                    nc.sync.dma_start(out=xt[:], in_=xv[:, jb])
                    xvts.append(xt)

            # k-blocks early (gate scores+exp), v-blocks late (AV filler).
            # xq half 0 split in two so qproj(0) can start ~2us earlier.
            nc.sync.dma_start(out=xqt[:, 0:4, 0:1024], in_=xq[:, 0, 0:4])
            fetch("k", 0)
            nc.sync.dma_start(out=xqt[:, 4:8, 0:1024], in_=xq[:, 0, 4:8])
            fetch("k", 1)
            fetch("k", 2)
            nc.sync.dma_start(out=xqt[:, :, 1024:2048], in_=xq[:, 1])
            fetch("v", 0)
            fetch("k", 3)
            fetch("v", 1)
            fetch("v", 2)
            fetch("v", 3)

            # ---- PE warm-keeper: spans preamble + xq0a DMA head so the
            # HAM clock gate is at 2.4 GHz when the first projection runs.
            wslot = slot("warm")
            for _ in range(46):
                nc.tensor.matmul(
                    wslot[0:128, 0:128], ident[:], ident[:],
                    start=True, stop=True, skip_group_check=True)

            # ---- AV queue: always-ready filler matmuls ----
            pts = [None] * NT           # per-tile exp(S^T) SBUF tiles
            av_ready = []
            av_bank_count = [0] * 4
            vdone = set()               # blocks whose vaug tiles exist

            def emit_av(n, qh_first=None):
                # drain up to n AV cells whose vaug tile exists; prefer
                # qh_first cells (used to unblock finalize of a q half)
                order = av_ready
                if qh_first is not None:
                    order = ([c for c in av_ready if c[1] == qh_first]
                             + [c for c in av_ready if c[1] != qh_first])
                emitted = []
                for cell in order:
                    if n <= 0:
                        break
                    t, qh = cell
                    if (t // 4) not in vdone:
                        continue
                    for sg in range(2):
                        seg = 2 * qh + sg
                        cnt = av_bank_count[seg]
                        nc.tensor.matmul(
                            oa[:, seg * 512:(seg + 1) * 512],
                            vaug[:, t, 0:65],
                            pts[t][:, seg * 512:(seg + 1) * 512],
                            start=(cnt == 0), stop=(cnt == NT - 1),
                            skip_group_check=True)
                        av_bank_count[seg] = cnt + 1
                    emitted.append(cell)
                    n -= 1
                for cell in emitted:
                    av_ready.remove(cell)

            def qproj(qh, c_lo=0, c_hi=EC, ps=None):
                # contraction chunks [c_lo, c_hi) accumulated into ps;
                # evacuate (add bq) when the last chunk group lands
                if ps is None:
                    ps = slot(f"pq{qh}")
                for c in range(c_lo, c_hi):
                    for sg in range(2):
                        nc.tensor.matmul(
                            ps[:, sg * 512:(sg + 1) * 512],
                            wqd_t[:, c, :],
                            xqt[:, c, qh * 1024 + sg * 512:
                                qh * 1024 + (sg + 1) * 512],
                            start=(c == 0), stop=(c == EC - 1),
                            skip_group_check=True)
                if c_hi == EC:
                    nc.vector.tensor_scalar_add(
                        qt[:, qh * 1024:(qh + 1) * 1024], ps[:], bq_t[:])
                return ps

            def proj_pass(specs):
                """One col-tiled projection pass. specs: list of
                ("k"|"v", jb, row0) with len 1 or 2; row0 in {0, 64} and
                distinct within a pass (col-tiling). k rows MUST equal
                (jb%2)*64 to match the kt layout; v rows are free."""
                ps = slot("pkv" + "_".join(f"{w}{j}" for w, j, _ in specs))
                for c in range(EC):
                    for which, jb, r0 in specs:
                        w = wk_t if which == "k" else wv_t
                        x = xkts[jb] if which == "k" else xvts[jb]
                        nc.tensor.matmul(
                            ps[r0:r0 + 64, 0:KB], w[:, c, :], x[:, c, :],
                            start=(c == 0), stop=(c == EC - 1),
                            skip_group_check=True)
                for which, jb, r0 in specs:
                    if which == "k":
                        assert r0 == (jb % 2) * 64
                        nc.vector.tensor_copy(
                            kt[r0:r0 + 64, jb * KB:(jb + 1) * KB],
                            ps[r0:r0 + 64, 0:KB])
                    else:
                        vtb = vtp.tile([128, KB], F16, tag="vt",
                                       name=f"vtb{jb}")
                        nc.vector.tensor_scalar_add(
                            vtb[r0:r0 + 64, :], ps[r0:r0 + 64, 0:KB],
                            bv_t[r0:r0 + 64])
                        # v^T [64, 512] -> vaug 4x[128, 64] via PE transpose
                        # (stays off the DMA rings; see v10 notes)
                        trs2 = psp.tile([128, 4, H], F16, tag="ps",
                                        name=f"vtr{jb}")
                        for j in range(4):
                            nc.tensor.transpose(
                                trs2[:, j, 0:H],
                                vtb[r0:r0 + 64, j * 128:(j + 1) * 128],
                                ident[r0:r0 + 64, r0:r0 + 64])
                        for j in range(4):
                            nc.vector.tensor_copy(
                                vaug[:, 4 * jb + j, 0:H], trs2[:, j, 0:H])
                        vdone.add(jb)

            def score_slabs(cells, av=0, qh_first=None):
                """Scores + exp for 1 or 2 (tile, qh) cells. A len-2 list
                must have opposite kt-half parity: the two cells' matmuls
                run concurrently via PE row tiling into different banks."""
                for t, qh in cells:
                    if pts[t] is None:
                        pts[t] = ptp.tile([128, S], F16, tag="pt", name=f"pt{t}")
                mms, exps = [], []
                for t, qh in cells:
                    g = ((t // 4) % 2) * 64
                    sl = slot(f"s{t}_{qh}")
                    for seg in range(2):
                        cs = slice(qh * 1024 + seg * 512,
                                   qh * 1024 + (seg + 1) * 512)
                        mms.append((sl, seg, g, t, cs))
                    exps.append((t, qh, sl))
                # interleave the two cells' matmuls seg-by-seg for pairing
                if len(cells) == 2:
                    mms = [mms[0], mms[2], mms[1], mms[3]]
                for sl, seg, g, t, cs in mms:
                    nc.tensor.matmul(
                        sl[:, seg * 512:(seg + 1) * 512],
                        kt[g:g + 64, t * 128:(t + 1) * 128], qt[g:g + 64, cs],
                        start=True, stop=True, skip_group_check=True)
                for t, qh, sl in exps:
                    nc.scalar.activation(
                        pts[t][:, qh * 1024:(qh + 1) * 1024], sl[:],
                        Exp, scale=0.125)
                    av_ready.append((t, qh))
                if av:
                    emit_av(av, qh_first=qh_first)

            # ---- finalize: transpose, normalize, store (out on sync ring)
            out_r = out[:].rearrange("(t p) h -> p t h", p=128)

            def finalize_chunk(cq):
                nc.vector.tensor_copy(
                    oasb[:, cq * 512:(cq + 1) * 512],
                    oa[:, cq * 512:(cq + 1) * 512])
                trs = psp.tile([128, 4, 66], F16, tag="ps", name=f"trs{cq}")
                for jj in range(4):
                    j = cq * 4 + jj
                    nc.tensor.transpose(
                        trs[:, jj, 0:65], oasb[:, j * 128:(j + 1) * 128],
                        ident[0:65, 0:65])
                rc = p5sb.tile([128, 4], F32, tag="rc", name=f"rc{cq}")
                nc.vector.reciprocal(rc[:], trs[:, :, 64])
                for jj in range(4):
                    j = cq * 4 + jj
                    nc.vector.tensor_scalar(
                        osb_all[:, j, :], trs[:, jj, 0:64], rc[:, jj:jj + 1],
                        None, op0=mybir.AluOpType.mult)
                nc.sync.dma_start(
                    out=out_r[:, cq * 4:(cq + 1) * 4, :],
                    in_=osb_all[:, cq * 4:(cq + 1) * 4, :])

            # ---- schedule (program order == per-engine issue order) ----
            # exp slab cadence ~1.15us; data arrival gates annotated.
            pq0 = qproj(0, 0, 4)               # xq0a
            proj_pass([("k", 0, 0)])           # k0
            qproj(0, 4, EC, pq0)               # xq0b
            score_slabs([(0, 0)])
            score_slabs([(1, 0)])
            score_slabs([(2, 0)])
            proj_pass([("k", 1, 64)])          # k1
            score_slabs([(3, 0)])
            score_slabs([(4, 0)])
            proj_pass([("k", 2, 0)])           # k2
            score_slabs([(5, 0)])
            score_slabs([(6, 0)])
            score_slabs([(7, 0)])
            score_slabs([(8, 0)])
            score_slabs([(9, 0)])
            score_slabs([(10, 0)])
            qproj(1)                           # xq1
            score_slabs([(11, 0)])
            score_slabs([(0, 1), (4, 1)])      # row-tiled pairs
            score_slabs([(1, 1), (5, 1)])
            proj_pass([("k", 3, 64), ("v", 0, 0)])   # col-tiled pair
            score_slabs([(2, 1), (6, 1)], av=2)
            score_slabs([(3, 1), (7, 1)], av=2)
            proj_pass([("v", 1, 0), ("v", 2, 64)])   # col-tiled pair
            score_slabs([(12, 0), (8, 1)], av=2)
            score_slabs([(13, 0), (9, 1)], av=2)
            proj_pass([("v", 3, 0)])
            score_slabs([(14, 0), (10, 1)], av=3)
            score_slabs([(15, 0), (11, 1)], av=3, qh_first=0)
            # all 16 qh0 cells exp'd; drain their AVs first so oa banks
            # 0/1 stop and chunks 0/1 finalize under the remaining exps
            emit_av(16, qh_first=0)
            finalize_chunk(0)
            score_slabs([(12, 1)], av=2)
            finalize_chunk(1)
            score_slabs([(13, 1)], av=2)
            score_slabs([(14, 1)], av=2)
            score_slabs([(15, 1)], av=2)
            emit_av(16)
            finalize_chunk(2)
            finalize_chunk(3)

    nc.finalize()
    return nc


def get_nc():
    if "nc" not in _CACHE:
        _CACHE["nc"] = _build_nc()
    return _CACHE["nc"]


def _stage_x(x, nblk, cb):
    # [S, E] f32 -> [128, nblk, EC, cb] f16 with [p, b, c, s] = x[b*cb+s, c*128+p]
    xt = np.ascontiguousarray(x.T.astype(np.float16))          # [E, S]
    xt = xt.reshape(EC, 128, nblk, cb).transpose(1, 2, 0, 3)   # [p, b, c, s]
    return np.ascontiguousarray(xt)


def make_in_maps(inputs):
    q = np.asarray(inputs["query"], np.float32)
    k = np.asarray(inputs["key_"], np.float32)
    v = np.asarray(inputs["value"], np.float32)
    wq_h = np.asarray(inputs["Wq"], np.float32).astype(np.float16)
    wqd_h = np.concatenate([wq_h, wq_h], axis=1)                # [E, 128]
    wqd_s = np.ascontiguousarray(
        wqd_h.reshape(EC, 128, 128).transpose(1, 0, 2))         # [128, EC, 128]
    wmats = {}
    for nm, key in (("wk", "Wk"), ("wv", "Wv")):
        w = np.asarray(inputs[key], np.float32).astype(np.float16)
        wmats[nm] = np.ascontiguousarray(
            w.reshape(EC, 128, H).transpose(1, 0, 2))           # [128, EC, H]
    bq = np.asarray(inputs["bq"], np.float32).reshape(H, 1)
    bv = np.asarray(inputs["bv"], np.float32).reshape(H, 1)
    bq_d = np.ascontiguousarray(np.tile(bq, (2, 1)))            # [128, 1]
    bv_d = np.ascontiguousarray(np.tile(bv, (2, 1)))
    in_maps = []
    for b in range(B):
        in_maps.append({
            "xq": _stage_x(q[b], 2, S // 2),
            "xk": _stage_x(k[b], NKB, KB),
            "xv": _stage_x(v[b], NKB, KB),
            "wqd": wqd_s, "wk": wmats["wk"], "wv": wmats["wv"],
            "bq": bq_d, "bv": bv_d,
        })
    return in_maps


def kernel(**inputs):
    nc = get_nc()
    in_maps = make_in_maps(inputs)
    res = run_bass_kernel_spmd(nc, in_maps, list(range(B)))
    return np.stack([res.results[b]["out"] for b in range(B)], axis=0)


# revision 14
# speedup vs baseline: 1.2842x; 1.2842x over previous
"""Trainium2 Bass kernel: single attention head (B=8, S=2048, E=1024, H=64).

Sharding: data-parallel over batch -- each of the 8 NeuronCores computes one
batch element's full attention. No collectives; every HBM byte read once.

v11 design (decoupled PSUM pools; exp-metronome; q-half-sequential AV):
  - v10 post-mortem: exp (the 35.6us ScalarE wall) gapped ~20us because
    every projection pass stole one of the TWO rotating f32 score slots
    (2 banks each), draining the score->exp pipeline for ~4us each time.
  - Fix: give projections their own PSUM pool. Banks come from making
    the AV accumulator q-half-sequential ([65, 1024] = 2 banks, recycled
    for the second half after the first half finalizes):
      2 rotating f32 score slots (4 banks) -- scores + finalize transposes
      2 projection slots (2 banks) -- all q/k/v projections + v-transposes
      1 AV accumulator (2 banks, sequential per q half)
    Projections never contend with the score rotation; exp runs
    back-to-back. (A 1-bank f16 slab variant is blocked: bass asserts
    non-transpose matmul output must be f32.)
  - xq staged seg-major ([128, half, seg, EC, 512]) and the stream starts
    xq0s0, k0, xq0s1: the first q seg-pass and kproj(0) run ~5us before
    the full q half has even landed; the first score cell runs seg-split
    (2x FD-512 exps) to start ScalarE ~2.5us earlier still.
  - Stream order (sync HWDGE ring is FIFO): xq0s0, k0, xq0s1, k1, k2,
    xq1s0, xq1s1, v0, k3, v1, v2, v3. k-blocks early (they gate exp),
    v-blocks late (AV matmuls are deferrable PE filler).
  - v^T -> vaug via PE transpose + DVE copy (off the DMA rings). Output
    DMAs on the sync ring; the scalar engine runs pure ACTIVATE in the
    critical window. Tail reordered: the last score slabs precede the
    qh0 AV flush so finalize overlaps the final exps.
  - As v9: duplicated-Wq stationary, transposed scores (keys on
    partitions), rowsums ride the ones column of the AV stationary, bk
    cancels in softmax, bq/bv fold into evacuations, finalize via PE
    transpose + DVE reciprocal-scale, batched f32 output DMA.
"""

import numpy as np

import concourse.bass as bass  # noqa: F401  (engine namespaces live on nc)
import concourse.mybir as mybir
import concourse.tile as tile
from concourse import bacc
from concourse.bass_utils import run_bass_kernel_spmd
from concourse.masks import make_identity

B, S, E, H = 8, 2048, 1024, 64
EC = E // 128    # contraction chunks (128 partitions each)
KB = 512         # kv block columns
NKB = S // KB    # 4 kv blocks
NT = S // 128    # key tiles
F16 = mybir.dt.float16
F32 = mybir.dt.float32

_CACHE = {}


def _build_nc():
    nc = bacc.Bacc(None)
    xq = nc.declare_dram_parameter("xq", [128, 2, 2, EC, KB], F16, isOutput=False)
    xk = nc.declare_dram_parameter("xk", [128, NKB, EC, KB], F16, isOutput=False)
    xv = nc.declare_dram_parameter("xv", [128, NKB, EC, KB], F16, isOutput=False)
    wqd = nc.declare_dram_parameter("wqd", [128, EC, 128], F16, isOutput=False)
    wk = nc.declare_dram_parameter("wk", [128, EC, H], F16, isOutput=False)
    wv = nc.declare_dram_parameter("wv", [128, EC, H], F16, isOutput=False)
    bq = nc.declare_dram_parameter("bq", [128, 1], F32, isOutput=False)
    bv = nc.declare_dram_parameter("bv", [128, 1], F32, isOutput=False)
    out = nc.declare_dram_parameter("out", [S, H], F32, isOutput=True)

    Exp = mybir.ActivationFunctionType.Exp

    with tile.TileContext(nc) as tc:
        with tc.tile_pool(name="const", bufs=1) as const, \
             tc.tile_pool(name="xkp", bufs=4) as xkp, \
             tc.tile_pool(name="xvp", bufs=4) as xvp, \
             tc.tile_pool(name="ptp", bufs=16) as ptp, \
             tc.tile_pool(name="vtp", bufs=2) as vtp, \
             tc.tile_pool(name="p5sb", bufs=2) as p5sb, \
             tc.tile_pool(name="psp", bufs=2, space="PSUM") as psp, \
             tc.tile_pool(name="pjp", bufs=2, space="PSUM") as pjp, \
             tc.tile_pool(name="oap", bufs=1, space="PSUM") as oap:

            # ---- constants on the scalar HWDGE ring (early, tiny) ----
            wqd_t = const.tile([128, EC, 128], F16, name="wqd_t")
            nc.scalar.dma_start(out=wqd_t[:], in_=wqd[:])
            wk_t = const.tile([128, EC, H], F16, name="wk_t")
            nc.scalar.dma_start(out=wk_t[:], in_=wk[:])
            wv_t = const.tile([128, EC, H], F16, name="wv_t")
            nc.scalar.dma_start(out=wv_t[:], in_=wv[:])
            bq_t = const.tile([128, 1], F32, name="bq_t")
            nc.scalar.dma_start(out=bq_t[:], in_=bq[:])
            bv_t = const.tile([128, 1], F32, name="bv_t")
            nc.scalar.dma_start(out=bv_t[:], in_=bv[:])

            qt = const.tile([128, S], F16, name="qt")     # q^T in BOTH halves
            kt = const.tile([128, S], F16, name="kt")     # k^T: half (jb%2)
            xqt = const.tile([128, 2, 2, EC, KB], F16, name="xqt")
            vaug = const.tile([128, NT, 80], F16, name="vaug")
            oasb = const.tile([65, S], F16, name="oasb")
            ident = const.tile([128, 128], F16, name="ident")
            osb_all = const.tile([128, NT, H], F32, name="osb_all")

            make_identity(nc, ident[:])
            nc.vector.memset(vaug[:, :, 64], 1.0)

            # AV accumulator: ONE q half at a time ([65, 1024] = 2 banks);
            # the qh1 tile reuses the banks after finalize of qh0 (bufs=1)
            oa_t = [None, None]
            oa_t[0] = oap.tile([65, S // 2], F32, tag="oa", name="oa0")

            # ---- input DMAs (sync HWDGE FIFO -- executes in this order) ----
            xkts, xvts = [], []

            def fetch(which, jb):
                if which == "k":
                    xt = xkp.tile([128, EC, KB], F16, tag="xk", name=f"xkt{jb}")
                    nc.sync.dma_start(out=xt[:], in_=xk[:, jb])
                    xkts.append(xt)
                else:
                    xt = xvp.tile([128, EC, KB], F16, tag="xv", name=f"xvt{jb}")
                    nc.sync.dma_start(out=xt[:], in_=xv[:, jb])
                    xvts.append(xt)

            nc.sync.dma_start(out=xqt[:, 0, 0], in_=xq[:, 0, 0])
            fetch("k", 0)
            nc.sync.dma_start(out=xqt[:, 0, 1], in_=xq[:, 0, 1])
            fetch("k", 1)
            fetch("k", 2)
            nc.sync.dma_start(out=xqt[:, 1, 0], in_=xq[:, 1, 0])
            nc.sync.dma_start(out=xqt[:, 1, 1], in_=xq[:, 1, 1])
            fetch("v", 0)
            fetch("k", 3)
            fetch("v", 1)
            fetch("v", 2)
            fetch("v", 3)

            # ---- PE warm-keeper: spans the preamble + xq0s0 DMA head so
            # the HAM clock gate is at 2.4 GHz for the first projection.
            wslot = psp.tile([128, 128], F32, tag="ps", name="warm")
            for _ in range(46):
                nc.tensor.matmul(
                    wslot[:], ident[:], ident[:],
                    start=True, stop=True, skip_group_check=True)

            # ---- AV queue: always-ready filler matmuls ----
            pts = [None] * NT           # per-tile exp(S^T) SBUF tiles
            av_ready = []
            av_bank_count = [0] * 4
            vdone = set()               # blocks whose vaug tiles exist

            def emit_av(n, half=None):
                # drain up to n AV cells whose vaug tile exists; only the
                # half whose oa accumulator currently exists is eligible
                emitted = []
                for cell in av_ready:
                    if n <= 0:
                        break
                    t, qh = cell
                    if (t // 4) not in vdone or oa_t[qh] is None:
                        continue
                    if half is not None and qh != half:
                        continue
                    for sg in range(2):
                        seg = 2 * qh + sg
                        cnt = av_bank_count[seg]
                        nc.tensor.matmul(
                            oa_t[qh][:, sg * 512:(sg + 1) * 512],
                            vaug[:, t, 0:65],
                            pts[t][:, qh * 1024 + sg * 512:
                                   qh * 1024 + (sg + 1) * 512],
                            start=(cnt == 0), stop=(cnt == NT - 1),
                            skip_group_check=True)
                        av_bank_count[seg] = cnt + 1
                    emitted.append(cell)
                    n -= 1
                for cell in emitted:
                    av_ready.remove(cell)

            def qproj_seg(qh, sg):
                # one 512-col q segment: 8 chunk matmuls into the proj
                # slot, then evacuate (+bq) to qt
                ps = pjp.tile([128, KB], F32, tag="pj", name=f"pq{qh}{sg}")
                for c in range(EC):
                    nc.tensor.matmul(
                        ps[:], wqd_t[:, c, :], xqt[:, qh, sg, c, :],
                        start=(c == 0), stop=(c == EC - 1),
                        skip_group_check=True)
                nc.vector.tensor_scalar_add(
                    qt[:, qh * 1024 + sg * 512:qh * 1024 + (sg + 1) * 512],
                    ps[:], bq_t[:])

            def proj_pass(specs):
                """One col-tiled k/v projection pass in the proj slot.
                specs: list of ("k"|"v", jb, row0), len 1 or 2; row0 in
                {0, 64}, distinct within a pass (col tiling). k rows MUST
                equal (jb%2)*64 to match kt; v rows are free."""
                ps = pjp.tile([128, KB], F32, tag="pj",
                              name="pkv" + "_".join(f"{w}{j}" for w, j, _ in specs))
                for c in range(EC):
                    for which, jb, r0 in specs:
                        w = wk_t if which == "k" else wv_t
                        x = xkts[jb] if which == "k" else xvts[jb]
                        nc.tensor.matmul(
                            ps[r0:r0 + 64, :], w[:, c, :], x[:, c, :],
                            start=(c == 0), stop=(c == EC - 1),
                            skip_group_check=True)
                vjobs = []
                for which, jb, r0 in specs:
                    if which == "k":
                        assert r0 == (jb % 2) * 64
                        nc.vector.tensor_copy(
                            kt[r0:r0 + 64, jb * KB:(jb + 1) * KB],
                            ps[r0:r0 + 64, :])
                    else:
                        vtb = vtp.tile([128, KB], F16, tag="vt",
                                       name=f"vtb{jb}")
                        nc.vector.tensor_scalar_add(
                            vtb[r0:r0 + 64, :], ps[r0:r0 + 64, :],
                            bv_t[r0:r0 + 64])
                        vjobs.append((jb, r0, vtb))
                for jb, r0, vtb in vjobs:
                    # v^T [64, 512] -> vaug 4x[128, 64] via PE transpose
                    # (stays off the DMA rings); trs2 reuses the proj slot
                    trs2 = pjp.tile([128, 4, H], F16, tag="pj",
                                    name=f"vtr{jb}")
                    for j in range(4):
                        nc.tensor.transpose(
                            trs2[:, j, 0:H],
                            vtb[r0:r0 + 64, j * 128:(j + 1) * 128],
                            ident[r0:r0 + 64, r0:r0 + 64])
                    for j in range(4):
                        nc.vector.tensor_copy(
                            vaug[:, 4 * jb + j, 0:H], trs2[:, j, 0:H])
                    vdone.add(jb)

            def score_seg(t, qh, sl, sg, exp=True):
                # one N=512 score matmul (+ optional FD-512 exp) for seg sg
                g = ((t // 4) % 2) * 64
                nc.tensor.matmul(
                    sl[:, sg * 512:(sg + 1) * 512],
                    kt[g:g + 64, t * 128:(t + 1) * 128],
                    qt[g:g + 64, qh * 1024 + sg * 512:
                       qh * 1024 + (sg + 1) * 512],
                    start=True, stop=True, skip_group_check=True)
                if exp:
                    nc.scalar.activation(
                        pts[t][:, qh * 1024 + sg * 512:
                               qh * 1024 + (sg + 1) * 512],
                        sl[:, sg * 512:(sg + 1) * 512], Exp, scale=0.125)

            def slab_tile(t, qh):
                if pts[t] is None:
                    pts[t] = ptp.tile([128, S], F16, tag="pt", name=f"pt{t}")
                return psp.tile([128, 1024], F32, tag="ps", name=f"s{t}_{qh}")

            def score_slabs(cells, av=0):
                """Scores + exp for 1 or 2 (tile, qh) cells: 2x N=512
                matmuls into a 2-bank f32 slab + one FD-1024 exp each.
                A len-2 list must have opposite kt-half parity; its four
                matmuls are interleaved seg-by-seg so the two cells run
                concurrently on the PE via row tiling."""
                work = [(t, qh, slab_tile(t, qh)) for t, qh in cells]
                for sg in range(2):
                    for t, qh, sl in work:
                        score_seg(t, qh, sl, sg, exp=False)
                for t, qh, sl in work:
                    nc.scalar.activation(
                        pts[t][:, qh * 1024:(qh + 1) * 1024], sl[:],
                        Exp, scale=0.125)
                    av_ready.append((t, qh))
                if av:
                    emit_av(av)

            # ---- finalize: transpose, normalize, store (out on sync ring)
            out_r = out[:].rearrange("(t p) h -> p t h", p=128)

            def finalize_chunk(cq):
                nc.vector.tensor_copy(
                    oasb[:, cq * 512:(cq + 1) * 512],
                    oa_t[cq // 2][:, (cq % 2) * 512:(cq % 2 + 1) * 512])
                trs = psp.tile([128, 4, 66], F16, tag="ps", name=f"trs{cq}")
                for jj in range(4):
                    j = cq * 4 + jj
                    nc.tensor.transpose(
                        trs[:, jj, 0:65], oasb[:, j * 128:(j + 1) * 128],
                        ident[0:65, 0:65])
                rc = p5sb.tile([128, 4], F32, tag="rc", name=f"rc{cq}")
                nc.vector.reciprocal(rc[:], trs[:, :, 64])
                for jj in range(4):
                    j = cq * 4 + jj
                    nc.vector.tensor_scalar(
                        osb_all[:, j, :], trs[:, jj, 0:64], rc[:, jj:jj + 1],
                        None, op0=mybir.AluOpType.mult)
                nc.sync.dma_start(
                    out=out_r[:, cq * 4:(cq + 1) * 4, :],
                    in_=osb_all[:, cq * 4:(cq + 1) * 4, :])

            # ---- schedule (program order == per-engine issue order) ----
            # exp slab cadence ~1.15us; stream arrival gates annotated.
            qproj_seg(0, 0)                    # xq0s0
            proj_pass([("k", 0, 0)])           # k0
            sl00 = slab_tile(0, 0)             # cell (0,0) split by seg:
            score_seg(0, 0, sl00, 0)           # exp starts before xq0s1
            qproj_seg(0, 1)                    # xq0s1
            score_seg(0, 0, sl00, 1)
            av_ready.append((0, 0))
            score_slabs([(1, 0)])
            proj_pass([("k", 1, 64)])          # k1
            score_slabs([(2, 0)])
            score_slabs([(3, 0)])
            proj_pass([("k", 2, 0)])           # k2
            score_slabs([(4, 0)])
            score_slabs([(5, 0)])
            score_slabs([(6, 0)])
            score_slabs([(7, 0)])
            qproj_seg(1, 0)                    # xq1s0
            score_slabs([(8, 0)])
            qproj_seg(1, 1)                    # xq1s1
            score_slabs([(9, 0)])
            score_slabs([(10, 0)])
            score_slabs([(11, 0)])
            proj_pass([("v", 0, 0)])           # v0
            score_slabs([(0, 1), (4, 1)])      # row-tiled pairs
            score_slabs([(1, 1), (5, 1)], av=2)
            proj_pass([("k", 3, 64)])          # k3
            score_slabs([(2, 1), (6, 1)], av=2)
            score_slabs([(3, 1), (7, 1)], av=2)
            proj_pass([("v", 1, 0), ("v", 2, 64)])   # col-tiled pair
            score_slabs([(12, 0), (8, 1)], av=2)
            score_slabs([(13, 0), (9, 1)], av=2)
            proj_pass([("v", 3, 0)])
            score_slabs([(14, 0), (10, 1)], av=3)
            score_slabs([(15, 0), (11, 1)], av=3)
            score_slabs([(12, 1)], av=2)
            score_slabs([(13, 1)], av=2)
            # all 16 qh0 AVs drained (emit_av auto-skips qh1 until its
            # accumulator exists) -> finalize chunks 0/1 under last exps,
            # then recycle the oa banks for the qh1 accumulator
            emit_av(4)
            finalize_chunk(0)
            finalize_chunk(1)
            oa_t[1] = oap.tile([65, S // 2], F32, tag="oa", name="oa1")
            score_slabs([(14, 1)], av=6)
            score_slabs([(15, 1)], av=6)
            emit_av(8)
            finalize_chunk(2)
            finalize_chunk(3)

    nc.finalize()
    return nc


def get_nc():
    if "nc" not in _CACHE:
        _CACHE["nc"] = _build_nc()
    return _CACHE["nc"]


def _stage_x(x, nblk, cb):
    # [S, E] f32 -> [128, nblk, EC, cb] f16 with [p, b, c, s] = x[b*cb+s, c*128+p]
    xt = np.ascontiguousarray(x.T.astype(np.float16))          # [E, S]
    xt = xt.reshape(EC, 128, nblk, cb).transpose(1, 2, 0, 3)   # [p, b, c, s]
    return np.ascontiguousarray(xt)


def _stage_xq(x):
    # [S, E] f32 -> [128, 2, 2, EC, 512] f16, seg-major:
    # [p, h, s, c, s2] = x[h*1024 + s*512 + s2, c*128 + p]
    xt = np.ascontiguousarray(x.T.astype(np.float16))          # [E, S]
    xt = xt.reshape(EC, 128, 2, 2, KB).transpose(1, 2, 3, 0, 4)
    return np.ascontiguousarray(xt)


def make_in_maps(inputs):
    q = np.asarray(inputs["query"], np.float32)
    k = np.asarray(inputs["key_"], np.float32)
    v = np.asarray(inputs["value"], np.float32)
    wq_h = np.asarray(inputs["Wq"], np.float32).astype(np.float16)
    wqd_h = np.concatenate([wq_h, wq_h], axis=1)                # [E, 128]
    wqd_s = np.ascontiguousarray(
        wqd_h.reshape(EC, 128, 128).transpose(1, 0, 2))         # [128, EC, 128]
    wmats = {}
    for nm, key in (("wk", "Wk"), ("wv", "Wv")):
        w = np.asarray(inputs[key], np.float32).astype(np.float16)
        wmats[nm] = np.ascontiguousarray(
            w.reshape(EC, 128, H).transpose(1, 0, 2))           # [128, EC, H]
    bq = np.asarray(inputs["bq"], np.float32).reshape(H, 1)
    bv = np.asarray(inputs["bv"], np.float32).reshape(H, 1)
    bq_d = np.ascontiguousarray(np.tile(bq, (2, 1)))            # [128, 1]
    bv_d = np.ascontiguousarray(np.tile(bv, (2, 1)))
    in_maps = []
    for b in range(B):
        in_maps.append({
            "xq": _stage_xq(q[b]),
            "xk": _stage_x(k[b], NKB, KB),
            "xv": _stage_x(v[b], NKB, KB),
            "wqd": wqd_s, "wk": wmats["wk"], "wv": wmats["wv"],
            "bq": bq_d, "bv": bv_d,
        })
    return in_maps


def kernel(**inputs):
    nc = get_nc()
    in_maps = make_in_maps(inputs)
    res = run_bass_kernel_spmd(nc, in_maps, list(range(B)))
    return np.stack([res.results[b]["out"] for b in range(B)], axis=0)
